# revision 1
# baseline (speedup 1.0000x reference)
"""Multi-head attention (b=4, n=2048, dim=1024, 16 heads x 64) on 8 Trainium2
NeuronCores.

Sharding: data-parallel over batch (4) x tensor-parallel over head-groups (2).
Each core gets one batch element and 8 heads: it computes its slice of the QKV
projection, full attention for its heads, and a partial output projection.
The host sums the two head-group partials per batch element and adds b_out.

Per-core pipeline (fp32 data; matmul-feeding tiles float32r):
  A:  per 512-wide n-chunk: PE-transpose x tiles into xT chunk tiles (SBUF),
      then qT = Wq^T x^T (staged to DRAM, streamed back in B), kT = Wk^T x^T
      (SBUF-resident, [inner, n] in 128-row strips) and v = x Wv (natural
      [n, inner], augmented with a ones column per head so the PV matmul also
      emits the softmax denominator).
  B:  i-blocks (ib) outer, heads inner: S^T j-tiles = matmul(lhsT=k^T_h
      j-block, rhs=q^T_h i-block) ([j, i] scores); exp on ScalarE
      (1/sqrt(dh) folded into the activation scale); PV matmul accumulates
      O_aug^T = v_aug^T @ P^T in PSUM half-blocks ([dh+1, 512]; last row =
      denominator). Tail: reciprocal of the denominator row, broadcast across
      partitions on GPSIMD, multiply -> normalized O^T strip. The PV matmuls
      trail S/exp by one step in a FIFO that carries across head boundaries,
      so ScalarE (the phase-B bottleneck) never runs dry.
  C:  y = O @ w_out via lhsT = O^T strips, emitted in single-PSUM-group
      slices woven through the NEXT i-block's heads (fills PE slack).
  A/B overlap: emission of B is interleaved into A's chunk emission as soon
  as the chunks a step needs are complete, so ScalarE starts exp work while
  the PE is still on the QKV projection.
"""

from contextlib import ExitStack

import numpy as np

import concourse.mybir as mybir
import concourse.tile as tile
from concourse import bacc, bass_utils

F32 = mybir.dt.float32
AF = mybir.ActivationFunctionType

# Full-problem constants (hardcoded per the harness contract).
B_FULL, N_FULL, DIM_FULL = 4, 2048, 1024
HEADS_FULL, DH = 16, 64
N_CORES = 8
GROUPS = 2                       # head-group (tensor-parallel) factor
HPC = HEADS_FULL // GROUPS       # heads per core = 8
INNER_PC = HPC * DH              # per-core inner dim = 512

# Matmul compute dtype: float32r streams 1 row/cycle (vs 4 for float32) at
# slightly reduced precision. All tiles feeding matmuls carry this dtype
# (producers round into it); numpy float32 maps onto it unchanged.
MM_DT = mybir.dt.float32r


def ts(i, size):
    return slice(i * size, (i + 1) * size)


def emit_core_kernel(nc, tc, x, wqkv, wout, y, *, n, dim, hpc, dh,
                     mm_dt=MM_DT, ib=1024, bcast="gpsimd", overlap=True,
                     s_lead=True):
    inner = hpc * dh
    KC = dim // 128          # contraction chunks for the qkv projection
    S = inner // 128         # 128-row strips of the per-core inner dim
    JT = n // 128            # key/value j-tiles
    NB = n // 512            # 512-wide n-chunks in phase A
    ib = min(ib, n)
    assert n % 512 == 0 and dim % 128 == 0 and inner % 128 == 0
    assert ib % 512 == 0 and n % ib == 0
    scale = float(1.0 / np.sqrt(dh))
    MD = mm_dt
    fc = min(512, dim)
    n_ibx = n // ib
    itpb = ib // 128                 # i-tiles per i-block
    state = {"chunk_done": -1}

    stack = ExitStack()
    with stack:
        const_pool = stack.enter_context(tc.tile_pool(name="const", bufs=1))
        persist = stack.enter_context(tc.tile_pool(name="persist", bufs=1))
        dram_pool = stack.enter_context(
            tc.tile_pool(name="dram", bufs=1, space="DRAM"))

        qt_dram = dram_pool.tile([S, 128, n], MD, name="qt_dram")

        # Constants are embedded in the NEFF and DMA'd in (no gpsimd on the
        # startup critical path). Anything that feeds a matmul is rounded
        # into mm_dt via DVE copies.
        ident = const_pool.tile([128, 128], F32, name="ident")
        nc.sync.dma_start(
            ident, nc.inline_tensor(np.eye(128, dtype=np.float32),
                             name=f'identc{nc.next_id()}').ap())
        oneshc = const_pool.tile([128, hpc], F32, name="oneshc")
        nc.sync.dma_start(
            oneshc, nc.inline_tensor(np.ones((128, hpc), np.float32),
                             name=f'onesc{nc.next_id()}').ap())
        if bcast == "matmul":
            ones_f32 = const_pool.tile([1, dh], F32, name="ones_f32")
            nc.gpsimd.memset(ones_f32, 1.0)
            ones_sb = const_pool.tile([1, dh], MD, name="ones_sb")
            nc.vector.tensor_copy(ones_sb, ones_f32)

        # Persistent SBUF tensors: kT strips, v_aug tiles, oT strip 0 (the
        # strip the first two heads write during the A/B overlap; strips 1+
        # are allocated after phase A's pools release).
        kT = []
        for s in range(S):
            kT.append(persist.tile([128, n], MD, name="kTs", tag=f"kT{s}"))
        v_sb = []
        for jt in range(JT):
            vt = persist.tile([128, hpc * (dh + 1)], MD, name="vts",
                              tag=f"v{jt}")
            v_sb.append(vt)
            nc.vector.tensor_copy(
                vt.rearrange("p (h c) -> p h c", c=dh + 1)[:, :, dh:dh + 1],
                oneshc.rearrange("p (h c) -> p h c", c=1))
        oT = []                      # strips allocated after phase A

        # q-block stream pool + loader live below the phase A pools so the
        # first q slices can prefetch while A is still running.
        qst_pool = stack.enter_context(tc.tile_pool(name="b_qst", bufs=2))
        qst_tiles = {}
        seq = [(bx, hh) for bx in range(n_ibx) for hh in range(hpc)]

        def qst_req(bx):
            return ((bx + 1) * ib - 1) // 512

        def load_qst(i):
            if i < len(seq) and i not in qst_tiles:
                bx, hh = seq[i]
                if qst_req(bx) > state["chunk_done"]:
                    return
                s2, r2 = divmod(hh * dh, 128)
                t = qst_pool.tile([128, ib], MD, name="qst")
                nc.sync.dma_start(
                    t[r2:r2 + dh, :], qt_dram[s2, r2:r2 + dh, ts(bx, ib)])
                qst_tiles[i] = t

        # ---- phase A pools ----
        actx = ExitStack()
        w_pool = actx.enter_context(tc.tile_pool(name="a_w", bufs=1))
        xin_pool = actx.enter_context(tc.tile_pool(name="a_xin", bufs=4))
        xts_pool = actx.enter_context(tc.tile_pool(name="a_xts", bufs=2))
        qstage_pool = actx.enter_context(
            tc.tile_pool(name="a_qstage", bufs=2))
        psT_pool = actx.enter_context(
            tc.tile_pool(name="a_psT", bufs=2, space="PSUM"))
        psA_pool = actx.enter_context(
            tc.tile_pool(name="a_ps", bufs=3, space="PSUM"))

        def emit_a():
            # First x tiles are on the critical path; their DMAs go first.
            # The very first tile arrives in column quarters so the first
            # transposes can start before the whole tile lands.
            first_x = []
            for j2 in range(4):
                x_in = xin_pool.tile([128, dim], F32, name="x_in")
                if j2 == 0:
                    for q in range(4):
                        nc.sync.dma_start(x_in[:, ts(q, dim // 4)],
                                          x[ts(j2, 128), ts(q, dim // 4)])
                else:
                    nc.sync.dma_start(x_in, x[ts(j2, 128), :])
                first_x.append(x_in)
            w_sb = []
            for kc in range(KC):
                wt = w_pool.tile([128, 3 * inner], MD, name="wt",
                                 tag=f"w{kc}")
                nc.sync.dma_start(wt, wqkv[ts(kc, 128), :])
                w_sb.append(wt)

            def transpose_ops(nb, xts):
                # lazily emitted (x_in DMA, transpose, copy) for one chunk
                for j2 in range(4):
                    it = nb * 4 + j2
                    if nb == 0:
                        x_in = first_x[j2]
                    else:
                        x_in = xin_pool.tile([128, dim], F32, name="x_in")
                        nc.sync.dma_start(x_in, x[ts(it, 128), :])
                    for kc in range(KC):
                        pt = psT_pool.tile([128, 128], F32, name="pt")
                        nc.tensor.transpose(pt, x_in[:, ts(kc, 128)], ident)
                        nc.vector.tensor_copy(xts[kc][:, ts(j2, 128)], pt)
                        yield None

            def alloc_xts():
                return [xts_pool.tile([128, 512], MD, name="xts",
                                      tag=f"xts{kc}") for kc in range(KC)]

            # transposes run one chunk ahead, woven between matmul groups
            # so the PE is never paced by the DVE copies draining PSUM.
            xts = alloc_xts()
            for _ in transpose_ops(0, xts):
                pass
            for nb in range(NB):
                if nb + 1 < NB:
                    xts_next = alloc_xts()
                    tq = transpose_ops(nb + 1, xts_next)
                else:
                    xts_next, tq = None, iter(())

                def weave(k=3):
                    for _ in range(k):
                        next(tq, None)

                # qT (to DRAM) / kT (SBUF) strips over this 512-wide chunk
                for which in (0, 1):
                    for s in range(S):
                        ps = psA_pool.tile([128, 512], F32, name="psA")
                        base = which * inner + s * 128
                        for kc in range(KC):
                            nc.tensor.matmul(
                                ps, w_sb[kc][:, base:base + 128], xts[kc],
                                start=(kc == 0), stop=(kc == KC - 1))
                        if which == 1:
                            nc.vector.tensor_copy(kT[s][:, ts(nb, 512)], ps)
                        else:
                            qs = qstage_pool.tile([128, 512], MD, name="qs")
                            nc.vector.tensor_copy(qs, ps)
                            nc.sync.dma_start(qt_dram[s, :, ts(nb, 512)], qs)
                        weave()
                # v natural: 4 row-tiles of 128 within this chunk
                for j2 in range(4):
                    it = nb * 4 + j2
                    ps = psA_pool.tile([128, inner], F32, name="psAv",
                                       tag="psAv")
                    for kc in range(KC):
                        nc.tensor.matmul(
                            ps, xts[kc][:, ts(j2, 128)],
                            w_sb[kc][:, 2 * inner:3 * inner],
                            start=(kc == 0), stop=(kc == KC - 1))
                    nc.vector.tensor_copy(
                        v_sb[it].rearrange(
                            "p (h c) -> p h c", c=dh + 1)[:, :, 0:dh],
                        ps.rearrange("p (h c) -> p h c", c=dh))
                    weave()
                for _ in tq:          # drain leftover transposes
                    pass
                xts = xts_next
                state["chunk_done"] = nb
                # prefetch the first q i-block slices as soon as their
                # chunks are staged, so phase B starts without a DMA wait
                if nb == qst_req(seq[0][0]):
                    load_qst(0)
                    load_qst(1)

        # ---- phase B/C emission (single generator; yields the A-chunk
        #      index the NEXT step needs before emitting it) ----
        wout_sb = []                  # filled after phase A pools release
        ysb_open = {}
        pend = []                     # (po, pexp, jt, h, ibx)
        proj_due = []

        def emit_tail(po_c, h, ibx, c):
            # normalize rows 0..dh-1 of one half-block by its denominator
            s_, r_ = divmod(h * dh, 128)
            recip_f = tail_pool.tile([1, 512], F32, name="recip_f")
            nc.vector.reciprocal(recip_f, po_c[dh:dh + 1, :])
            bc = tail_pool.tile([dh, 512], F32, name="bc")
            if bcast == "gpsimd":
                nc.gpsimd.partition_broadcast(bc, recip_f)
            else:
                recip = tail_pool.tile([1, 512], MD, name="recip")
                nc.vector.tensor_copy(recip, recip_f)
                pb = psB_pool.tile([dh, 512], F32, name="pb")
                nc.tensor.matmul(pb, ones_sb, recip, start=True, stop=True)
                nc.vector.tensor_copy(bc, pb)
            off = ibx * ib + c * 512
            nc.vector.tensor_mul(
                oT[s_][r_:r_ + dh, off:off + 512], po_c[0:dh, :], bc)

        def emit_proj_group(it, c, final=False):
            # one PSUM-group slice of the projection for i-tile `it`. The
            # final flush borrows the (by then idle) psS slots so its groups
            # pipeline instead of serializing on the single psC bank.
            if c == 0:
                ysb_open[it] = y_pool.tile([128, dim], F32, name="ysb")
            ysb = ysb_open[it]
            if final:
                ps = psS_pool.tile([128, fc], F32, name="psS")
            else:
                ps = psC_pool.tile([128, fc], F32, name="psC")
            for t in range(S):
                nc.tensor.matmul(
                    ps, oT[t][:, ts(it, 128)], wout_sb[t][:, ts(c, fc)],
                    start=(t == 0), stop=(t == S - 1))
            nc.vector.tensor_copy(ysb[:, ts(c, fc)], ps)
            if c == dim // fc - 1:
                nc.sync.dma_start(y[ts(it, 128), :], ysb)
                del ysb_open[it]

        def pop_pend():
            po, pexp, jt, h, ibx = pend.pop(0)
            vcol = slice(h * (dh + 1), (h + 1) * (dh + 1))
            for c in range(ib // 512):
                nc.tensor.matmul(
                    po[c], v_sb[jt][:, vcol], pexp[:, ts(c, 512)],
                    start=(jt == 0), stop=(jt == JT - 1))
            if jt == JT - 1:
                for c in range(ib // 512):
                    emit_tail(po[c], h, ibx, c)

        head_state = {}               # gi -> (qTh, kTh, po)

        def emit_s(k, steps):
            # S matmuls for flat step k (allocates the head's tiles on its
            # first step; runs one step AHEAD of exp so ScalarE never waits
            # on the PE at head boundaries)
            ibx, h, jt = steps[k]
            gi = ibx * hpc + h
            if jt == 0:
                load_qst(gi)
                load_qst(gi + 1)
                s_, r_ = divmod(h * dh, 128)
                head_state[gi] = (
                    qst_tiles.pop(gi)[r_:r_ + dh, :],
                    kT[s_][r_:r_ + dh, :],
                    [psO_pool.tile([dh + 1, 512], F32, name="po")
                     for _ in range(ib // 512)])
            qTh, kTh, po = head_state[gi]
            psS = psS_pool.tile([128, ib], F32, name="psS")
            for c in range(ib // 512):
                nc.tensor.matmul(
                    psS[:, ts(c, 512)], kTh[:, ts(jt, 128)],
                    qTh[:, ts(c, 512)], start=True, stop=True)
            return psS, po

        def b_emit():
            nonlocal proj_due
            steps = [(bx, hh, jt) for bx in range(n_ibx)
                     for hh in range(hpc) for jt in range(JT)]
            s_ahead = None            # (psS, po) for step k, S already done
            for k, (ibx, h, jt) in enumerate(steps):
                yield max(qst_req(ibx), jt // 4)
                if s_ahead is None:
                    s_ahead = emit_s(k, steps)
                psS, po = s_ahead
                nx = k + 1
                if s_lead:
                    # S for step k+1 goes out before exp(k)
                    if nx < len(steps) and (steps[nx][2] // 4 <=
                                            state["chunk_done"]) and \
                            qst_req(steps[nx][0]) <= state["chunk_done"]:
                        s_ahead = emit_s(nx, steps)
                    else:
                        s_ahead = None
                pexp = pexp_pool.tile([128, ib], MD, name="pexp")
                nc.scalar.activation(pexp, psS, AF.Exp, scale=scale)
                if not s_lead:
                    if nx < len(steps) and (steps[nx][2] // 4 <=
                                            state["chunk_done"]) and \
                            qst_req(steps[nx][0]) <= state["chunk_done"]:
                        s_ahead = emit_s(nx, steps)
                    else:
                        s_ahead = None
                pend.append((po, pexp, jt, h, ibx))
                while len(pend) > 1:
                    pop_pend()
                if jt == JT - 1:
                    head_state.pop(ibx * hpc + h, None)
                if jt == 0:
                    spread_n = -(-len(proj_due) // hpc) + 1
                if (proj_due and jt % max(1, JT // max(1, spread_n)) ==
                        JT // max(1, spread_n) - 1 and jt < JT - 1):
                    emit_proj_group(*proj_due.pop(0))
                if jt == JT - 1 and h == hpc - 1:
                    # end of an i-block
                    while proj_due:
                        emit_proj_group(*proj_due.pop(0))
                    proj_due = [(it, c)
                                for it in range(ibx * itpb, (ibx + 1) * itpb)
                                for c in range(dim // fc)]
                    if ibx == n_ibx - 1:
                        while pend:
                            pop_pend()
                        for it, c in proj_due:
                            emit_proj_group(it, c, final=True)
                        proj_due = []

        # ---- drive phase A, then phase B/C ----
        emit_a()
        actx.close()      # release phase A pools

        # phase B/C pools live in the space freed by phase A
        with (
            tc.tile_pool(name="b_psS", bufs=2, space="PSUM") as psS_pool_,
            tc.tile_pool(name="b_psO", bufs=3, space="PSUM") as psO_pool_,
            tc.tile_pool(name="c_ps", bufs=1, space="PSUM") as psC_pool,
            tc.tile_pool(name="b_pexp", bufs=4) as pexp_pool,
            tc.tile_pool(name="b_tail", bufs=3) as tail_pool,
            tc.tile_pool(name="c_w", bufs=1) as wout_pool,
            tc.tile_pool(name="c_y", bufs=3) as y_pool,
        ):
            psS_pool, psO_pool = psS_pool_, psO_pool_
            if bcast == "matmul":
                psB_pool = stack.enter_context(
                    tc.tile_pool(name="b_psB", bufs=1, space="PSUM"))
            for s in range(S):
                oT.append(persist.tile([128, n], MD, name="oTs",
                                       tag=f"oT{s}"))
            for t in range(S):
                wo = wout_pool.tile([128, dim], MD, name="wo", tag=f"wo{t}")
                nc.sync.dma_start(wo, wout[ts(t, 128), :])
                wout_sb.append(wo)
            for _ in b_emit():
                pass


_BUILD_CACHE = {}


def build_nc(n=N_FULL, dim=DIM_FULL, hpc=HPC, dh=DH, mm_dt=MM_DT, ib=1024,
             bcast="gpsimd", overlap=True, reps=1, s_lead=True):
    key = (n, dim, hpc, dh, str(mm_dt), ib, bcast, overlap, reps, s_lead)
    if key in _BUILD_CACHE:
        return _BUILD_CACHE[key]
    inner = hpc * dh
    nc = bacc.Bacc("TRN2", target_bir_lowering=False, debug=False)
    x = nc.dram_tensor("x", [n, dim], F32, kind="ExternalInput").ap()
    wqkv = nc.dram_tensor("w_qkv", [dim, 3 * inner], mm_dt,
                          kind="ExternalInput").ap()
    wout = nc.dram_tensor("w_out", [inner, dim], mm_dt,
                          kind="ExternalInput").ap()
    y = nc.dram_tensor("y", [n, dim], F32, kind="ExternalOutput").ap()
    with tile.TileContext(nc) as tc:
        with nc.allow_low_precision(
                reason="float32r is 4-byte; PSUM accumulation stays fp32"):
            for _ in range(reps):
                emit_core_kernel(nc, tc, x, wqkv, wout, y, n=n, dim=dim,
                                 hpc=hpc, dh=dh, mm_dt=mm_dt, ib=ib,
                                 bcast=bcast, overlap=overlap, s_lead=s_lead)
    nc.compile()
    _BUILD_CACHE[key] = nc
    return nc


def make_in_maps(x, w_qkv, w_out):
    """Shard full inputs into the 8 per-core input maps."""
    x = np.asarray(x, dtype=np.float32)
    w_qkv = np.asarray(w_qkv, dtype=np.float32)
    w_out = np.asarray(w_out, dtype=np.float32)
    qk_off = HEADS_FULL * DH          # 1024: start of K block in w_qkv
    in_maps = []
    for c in range(N_CORES):
        b, g = divmod(c, GROUPS)
        cols = ts(g, INNER_PC)
        wq = w_qkv[:, cols]
        wk = w_qkv[:, qk_off + g * INNER_PC: qk_off + (g + 1) * INNER_PC]
        wv = w_qkv[:, 2 * qk_off + g * INNER_PC: 2 * qk_off + (g + 1) * INNER_PC]
        in_maps.append({
            "x": np.ascontiguousarray(x[b]),
            "w_qkv": np.ascontiguousarray(np.concatenate([wq, wk, wv], axis=1)),
            "w_out": np.ascontiguousarray(w_out[cols, :]),
        })
    return in_maps


def kernel(x, w_qkv, w_out, b_out, trace=False):
    b_out = np.asarray(b_out, dtype=np.float32)
    nc = build_nc()
    in_maps = make_in_maps(x, w_qkv, w_out)
    res = bass_utils.run_bass_kernel_spmd(
        nc, in_maps, core_ids=list(range(N_CORES)), trace=trace)
    ys = [r["y"] for r in res.results]
    out = np.empty((B_FULL, N_FULL, DIM_FULL), dtype=np.float32)
    for b in range(B_FULL):
        out[b] = ys[GROUPS * b] + ys[GROUPS * b + 1] + b_out[None, :]
    if trace:
        kernel.last_result = res
    return out



# revision 92
# speedup vs baseline: 1.1112x; 1.1112x over previous
"""Multi-head attention (b=4, n=2048, dim=1024, 16 heads x 64) on 8 Trainium2
NeuronCores.

Sharding: data-parallel over batch (4) x tensor-parallel over head-groups (2).
Each core gets one batch element and 8 heads; the host sums the two
head-group partials per batch element and adds b_out.

Per-core pipeline (bf16 matmul feeds, fp32 PSUM):
  A (woven into B):  x tiles DMA'd fp32, PE-transposed (bf16 identity ->
      1 cy/row) into xts bf16; QKV strips computed as half-width PSUM
      groups through a single transient PSUM bank and copied to
      SBUF-resident qT / kT strips and v tiles (all bf16).  A-items are
      emitted lazily between B steps so the Activation engine starts exp
      work ~10us into the kernel.
  B:  steps ordered (i-block, head-PAIR, jt, head): S^T j-tiles =
      matmul(lhsT=kT_h j-block, rhs=qT_h i-block) -> psS [128, ib];
      exp on ScalarE (scale=1/sqrt(dh)) -> pexp bf16; PV in the
      output-natural layout po[i, dh] (lhsT=pexp i-slices, rhs=v bf16,
      N=dh -> full 128-partition output fill, half the PE time of the
      [dh, i] layout); denominators via N=1 matmuls into a dedicated
      PSUM bank.  PSUM accumulation groups share banks via the
      hardware zero-region semantics: only the first matmul of the
      first group in a bank uses start=True, later groups ride the
      pending-zero region (skip_group_check).  PV trails exp through a
      FIFO that grows while A hasn't delivered the needed v chunk yet,
      so ScalarE never stalls on phase A.
      Tail per head: DVE reciprocal of the denominator column and a
      fused normalize+copy (tensor_scalar mult by a per-partition
      scalar) -> O_sb; per pair, PE transposes O_sb back to oT strips.
  C:  y = O @ w_out via lhsT = oT strips, woven through the NEXT
      i-block's steps; PSUM->SBUF output copies ride the (otherwise
      idle) Pool engine; final flush pipelines through the psS slots.
"""

from contextlib import ExitStack

import numpy as np
import ml_dtypes

import concourse.mybir as mybir
import concourse.tile as tile
from concourse import bacc, bass_utils

F32 = mybir.dt.float32
BF16 = mybir.dt.bfloat16
AF = mybir.ActivationFunctionType
MUL = mybir.AluOpType.mult

# Full-problem constants (hardcoded per the harness contract).
B_FULL, N_FULL, DIM_FULL = 4, 2048, 1024
HEADS_FULL, DH = 16, 64
N_CORES = 8
GROUPS = 2                       # head-group (tensor-parallel) factor
HPC = HEADS_FULL // GROUPS       # heads per core = 8
INNER_PC = HPC * DH              # per-core inner dim = 512


def ts(i, size):
    return slice(i * size, (i + 1) * size)


def emit_core_kernel(nc, tc, x, wqkv, wout, y, *, n, dim, hpc, dh, ib=1024):
    inner = hpc * dh
    KC = dim // 128          # contraction chunks for the qkv projection
    S = inner // 128         # 128-row strips of the per-core inner dim
    JT = n // 128            # key/value j-tiles
    NB = n // 512            # 512-wide n-chunks in phase A
    PAIRS = hpc // 2
    ib = min(ib, n)
    n_ibx = n // ib
    itpb = ib // 128                 # i-tiles per i-block
    cpb = ib // 512                  # 512-chunks per i-block
    assert n % 512 == 0 and dim % 128 == 0 and inner % 128 == 0
    assert ib % 512 == 0 and n % ib == 0 and hpc % 2 == 0
    scale = float(1.0 / np.sqrt(dh))
    fc = min(512, dim)
    ncs = dim // fc                  # projection col slices per i-tile

    stack = ExitStack()
    with stack:
        const_pool = stack.enter_context(tc.tile_pool(name="const", bufs=1))
        persist = stack.enter_context(tc.tile_pool(name="persist", bufs=1))

        # ---- constants ----
        ident_f = const_pool.tile([128, 128], F32, name="ident_f")
        ident_src = nc.inline_tensor(np.eye(128, dtype=np.float32),
                                     name=f'identc{nc.next_id()}').ap()
        nc.scalar.dma_start(ident_f, ident_src)
        ident = const_pool.tile([128, 128], BF16, name="ident")
        nc.vector.tensor_copy(ident, ident_f)
        ident_r = ident_f.bitcast(mybir.dt.float32r)
        ones_col = const_pool.tile([128, 1], BF16, name="ones_col")
        nc.gpsimd.memset(ones_col, 1.0)

        # ---- persistent SBUF tensors ----
        kT = [persist.tile([128, n], BF16, name="kTs", tag=f"kT{s}")
              for s in range(S)]
        qT = [persist.tile([128, n], BF16, name="qTs", tag=f"qT{s}")
              for s in range(S)]
        oT = [persist.tile([128, n], BF16, name="oTs", tag=f"oT{s}")
              for s in range(S)]
        v_sb = [persist.tile([128, inner], BF16, name="vts", tag=f"v{jt}")
                for jt in range(JT)]
        # xtsF[kc]: transposed x (all n), contraction rows kc*128
        xtsF = [persist.tile([128, n], BF16, name="xts", tag=f"xts{kc}")
                for kc in range(KC)]
        w_sb = [persist.tile([128, 3 * inner], BF16, name="wt",
                             tag=f"w{kc}") for kc in range(KC)]
        wout_sb = [persist.tile([128, dim], BF16, name="wo", tag=f"wo{t}")
                   for t in range(S)]

        pexp_pool = stack.enter_context(tc.tile_pool(name="b_pexp",
                                                     bufs=14))
        osb_pool = stack.enter_context(tc.tile_pool(name="b_osb", bufs=2))
        tail_pool = stack.enter_context(tc.tile_pool(name="b_tail", bufs=4))
        y_pool = stack.enter_context(tc.tile_pool(name="c_y", bufs=3))

        ps_pool = stack.enter_context(
            tc.tile_pool(name="ps", bufs=1, space="PSUM"))
        # psS: 2 x [128, ib] (4 banks with ib=1024); po: 2 x [128, 512]
        # (1 bank each); den: [128, 128] (1 bank); psC: [128, 512] (1 bank).
        den_ps = ps_pool.tile([128, 128], F32, name="den_ps", tag="den")
        psC = ps_pool.tile([128, 512], F32, name="psC", tag="psC")

        # weight DMAs are emitted inside a_items (interleaved with x) so
        # the SP sequencer's ~650ns-per-DMA issue cost doesn't serialize
        # the startup critical path.

        # ---------- transient-PSUM bank ring ----------
        # The tile framework tracks PSUM deps at whole-tile granularity, so
        # consecutive transient groups in ONE bank serialize against each
        # other's drain copies.  During the prefix (before any PV/den
        # accumulation), the idle po banks join psC in a rotation, which
        # fully pipelines the x-transposes and first qkv strips.  Once B
        # starts, transients use only psC and are spread across the
        # Act-bound steps so the drain latency hides.
        ring = {"banks": [psC], "i": 0, "r": 0}

        def set_ring(banks):
            ring["banks"] = banks
            ring["i"] = 0
            ring["r"] = 0
            ring["cur"] = banks[0]

        def next_bank():
            b = ring["banks"][ring["i"] % len(ring["banks"])]
            ring["i"] += 1
            return b

        def next_tregion(dt=BF16):
            # one [128, 128] transpose slot; a fresh BANK every call —
            # whole-tile dep tracking makes same-bank reuse serialize
            # against the previous op's drain copy
            w = 64 if dt == BF16 else 128
            return next_bank()[:, 0:w].bitcast(dt)

        # ---------- phase A as a lazily-pulled item stream ----------
        adone = {"qT": set(), "kT": set(), "v": set(), "xts": set()}
        cp_state = {"act_free": True, "flip": 0}

        def cp(dst, src, allow_act=True):
            # before the exp stream starts (and after it ends) the Act
            # engine is idle; route alternate drain copies there
            if cp_state["act_free"] and allow_act:
                cp_state["flip"] ^= 1
                if cp_state["flip"]:
                    nc.scalar.copy(dst, src)
                    return
            nc.vector.tensor_copy(dst, src)

        def a_items():
            dq_state = {"i": 0}

            def dq():
                # all phase-A loads ride the SP queue back-to-back: same-
                # queue DMAs pipeline at the ~650ns issue rate, while
                # cross-queue interleaving serializes on completion order
                return nc.sync

            def fetch_xts(c0, c1, kcs=None):
                # XBAR DMA transpose: x[c0..c1 chunks, kc cols] ->
                # xtsF[kc][:, span] — one DMA per kc spanning several
                # chunks (the shared HWDGE costs ~625ns per DMA, so fewer
                # bigger transfers win)
                c1 = min(c1, NB)
                if c0 >= c1:
                    return
                for kc in (range(KC) if kcs is None else kcs):
                    dq().dma_start_transpose(
                        xtsF[kc][:, c0 * 512: c1 * 512],
                        x[c0 * 512: c1 * 512, ts(kc, 128)])
                for c in range(c0, c1):
                    adone["xts"].add(c)

            # PE warmup: dummy transposes keep the array busy from t~2us
            # so the first strips run at full p-state (idle drops the PE
            # to 2-4x slower p-states); results are never read.
            for i in range(220):
                nc.tensor.matmul(psC[:, 0:64].bitcast(BF16), ident, ident,
                                 is_transpose=True, skip_group_check=True)

            # startup DMA order: all first-half x^T transposes, then all
            # q/k weight strips — same-type DMAs pipeline at the ~650ns
            # issue rate while mixed types serialize on completion
            fetch_xts(0, min(2, NB))
            for kc in range(KC):
                dq().dma_start(w_sb[kc][:, 0:2 * inner],
                               wqkv[ts(kc, 128), 0:2 * inner])

            def transposes(c):
                if c < NB and c not in adone["xts"]:
                    fetch_xts(c, c + 2)
                    yield 100

            def qk_strip(which, s, c):
                # qT/kT strip s over one 512-chunk: one full-bank group
                base = which * inner + s * 128
                dst = (qT if which == 0 else kT)[s]
                ring["r"] = 0
                slot = next_bank()
                for kc in range(KC):
                    nc.tensor.matmul(
                        slot[:, 0:512], w_sb[kc][:, base:base + 128],
                        xtsF[kc][:, ts(c, 512)],
                        start=(kc == 0), stop=(kc == KC - 1),
                        skip_group_check=True)
                cp(dst[:, ts(c, 512)], slot[:, 0:512],
                   allow_act=(s == 0 and c < cpb))
                yield KC * 215
                adone["qT" if which == 0 else "kT"].add((s, c))

            def v_tile(c, j2):
                it = c * 4 + j2
                ring["r"] = 0
                slot = next_bank()
                for kc in range(KC):
                    nc.tensor.matmul(
                        slot[:, 0:inner],
                        xtsF[kc][:, c * 512 + j2 * 128:
                                  c * 512 + (j2 + 1) * 128],
                        w_sb[kc][:, 2 * inner: 3 * inner],
                        start=(kc == 0), stop=(kc == KC - 1),
                        skip_group_check=True)
                nc.vector.tensor_copy(v_sb[it], slot[:, 0:inner])
                yield KC * inner * 0.42
                if j2 == 3:
                    adone["v"].add(c)

            # demand-ordered: pair 0's q/k first (kT needs every chunk
            # within the first pair-block), then v chunks interleaved
            # with later pairs' strips; later-i-block q chunks last.
            # Chunk c+1's transposes interleave with chunk c's strips so
            # the PE strip work overlaps the DVE/Act transpose drains.
            tq = {c: transposes(c) for c in range(NB)}

            def t_weave(c, k2=2):
                if c < NB:
                    for _ in range(k2):
                        if next(tq[c], "E") != "E":
                            yield 430

            if NB >= 2 and cpb == 2 and len(ring["banks"]) >= 5:
                # fast prefix: the four s0 strip groups accumulate
                # kc-major on dedicated banks, paced by the w arrivals
                sbank = ring["banks"][1:5]
                descs = [(0, 0), (1, 0), (0, 1), (1, 1)]  # (which, c)
                for kc in range(KC):
                    for g, (which, c) in enumerate(descs):
                        base = which * inner
                        nc.tensor.matmul(
                            sbank[g][:, 0:512],
                            w_sb[kc][:, base:base + 128],
                            xtsF[kc][:, ts(c, 512)],
                            start=(kc == 0), stop=(kc == KC - 1),
                            skip_group_check=True)
                    yield 860
                tq[0] = tq[1] = iter(())
                for g, (which, c) in enumerate(descs):
                    dst = (qT if which == 0 else kT)[0]
                    cp(dst[:, ts(c, 512)], sbank[g][:, 0:512])
                    adone["qT" if which == 0 else "kT"].add((0, c))
                yield 200
                fetch_xts(2, NB)
                for t in range(S):
                    nc.sync.dma_start(wout_sb[t], wout[ts(t, 128), :])
                for kc in range(KC):
                    nc.sync.dma_start(
                        w_sb[kc][:, 2 * inner: 3 * inner],
                        wqkv[ts(kc, 128), 2 * inner: 3 * inner])
                for c in range(2, NB):
                    yield from qk_strip(1, 0, c)
                for c in range(min(2, NB)):
                    for j2 in range(4):
                        yield from v_tile(c, j2)
            else:
                yield from tq[0]
                for c in range(NB):
                    if c < cpb:
                        yield from t_weave(c + 1)
                        yield from qk_strip(0, 0, c)
                    yield from t_weave(c + 1)
                    yield from qk_strip(1, 0, c)
                    yield from t_weave(c + 1, 4)
                    if c == min(1, NB - 1):
                        for t in range(S):
                            nc.sync.dma_start(wout_sb[t],
                                              wout[ts(t, 128), :])
            strip_q = [("q", s, c) for s in range(1, S) for c in range(cpb)]
            strip_k = [("k", s, c) for s in range(1, S) for c in range(NB)]
            later = []
            for s in range(1, S):
                later.extend([("q", s, c) for c in range(cpb)])
                later.extend([("k", s, c) for c in range(NB)])
            if not any(2 * inner <= 3 * inner and c in adone["v"]
                       for c in range(NB)):
                for kc in range(KC):
                    nc.sync.dma_start(
                        w_sb[kc][:, 2 * inner: 3 * inner],
                        wqkv[ts(kc, 128), 2 * inner: 3 * inner])
            for c in range(NB):
                if c in adone["v"]:
                    continue
                for j2 in range(4):
                    yield from v_tile(c, j2)
                take, later = later[:cpb + 1], later[cpb + 1:]
                for kind, s, c2 in take:
                    yield from qk_strip(0 if kind == "q" else 1, s, c2)
            for kind, s, c2 in later:
                yield from qk_strip(0 if kind == "q" else 1, s, c2)
            for s in range(S):
                for c in range(cpb, min(NB, cpb * n_ibx)):
                    if (s, c) not in adone["qT"]:
                        yield from qk_strip(0, s, c)

        agen = a_items()
        a_exhausted = {"v": False}
        a_budget = {"ns": 0.0}

        def pull_a():
            """Pull one A item; returns False when exhausted."""
            cost = next(agen, None)
            if cost is None:
                a_exhausted["v"] = True
                return False
            a_budget["ns"] -= cost
            return True

        def pull_paced():
            if a_budget["ns"] > 0 and not a_exhausted["v"]:
                pull_a()

        def pull_until(pred):
            while not pred():
                if not pull_a():
                    assert pred(), "phase A exhausted without satisfying dep"
                    return

        # ---------- phase B/C emission ----------
        # steps ordered (ibx, pair, jt, h-in-pair)
        steps = [(ibx, 2 * p + hi, jt)
                 for ibx in range(n_ibx) for p in range(PAIRS)
                 for jt in range(JT) for hi in range(2)]

        def s_req(k):
            ibx, h, jt = steps[k]
            s = h // 2
            return (all((s, c) in adone["qT"]
                        for c in range(ibx * cpb, (ibx + 1) * cpb))
                    and (s, jt // 4) in adone["kT"])

        pend = []                 # FIFO of (pexp, ibx, h, jt)
        po_tiles = {}             # h -> po tile for the live pair
        trans_due = []            # deferred (pair, ibx, it, osb) transposes
        proj_due = []
        osb_tiles = {}

        def emit_s(k):
            ibx, h, jt = steps[k]
            s, r = divmod(h * dh, 128)
            psS = ps_pool.tile([128, ib], F32, name="psS", tag="psS",
                               bufs=2)
            for c in range(cpb):
                nc.tensor.matmul(
                    psS[:, ts(c, 512)], kT[s][r:r + dh, ts(jt, 128)],
                    qT[s][r:r + dh, ibx * ib + c * 512: ibx * ib + (c + 1) * 512],
                    start=True, stop=True)
            return psS

        def pop_pend():
            if ring["banks"][0] is not psC or len(ring["banks"]) > 1:
                set_ring([psC])      # PV/den claim the po banks now
            pexp, ibx, h, jt = pend.pop(0)
            if jt == 0:
                po_tiles[h] = ps_pool.tile(
                    [128, itpb * dh], F32, name="po", tag=f"po{h % 2}")
            po = po_tiles[h]
            first_of_pair = (h % 2 == 0 and jt == 0)
            for it in range(itpb):
                lhsT = pexp[:, ts(it, 128)]
                nc.tensor.matmul(
                    po[:, ts(it, dh)], lhsT, v_sb[jt][:, ts(h, dh)],
                    start=(jt == 0 and it == 0), stop=(jt == JT - 1),
                    skip_group_check=True)
                nc.tensor.matmul(
                    den_ps[:, (ibx % 2) * 64 + (h % 2) * itpb + it:
                           (ibx % 2) * 64 + (h % 2) * itpb + it + 1],
                    lhsT, ones_col,
                    start=(first_of_pair and it == 0), stop=(jt == JT - 1),
                    skip_group_check=True)
            if jt == JT - 1:
                emit_tail(ibx, h)

        def emit_tail(ibx, h):
            po = po_tiles.pop(h)
            p = h // 2
            for it in range(itpb):
                dcol = (ibx % 2) * 64 + (h % 2) * itpb + it
                recip = tail_pool.tile([128, 1], F32, name="recip")
                nc.vector.reciprocal(recip, den_ps[:, dcol:dcol + 1])
                if h % 2 == 0:
                    osb_tiles[it] = osb_pool.tile([128, 128], BF16,
                                                  name="osb", tag=f"osb{it}")
                nc.vector.tensor_scalar(
                    osb_tiles[it][:, (h % 2) * dh: (h % 2 + 1) * dh],
                    po[:, ts(it, dh)], recip, None, MUL)
            if h % 2 == 1:
                for it in range(itpb):
                    trans_due.append((p, ibx, it, osb_tiles.pop(it)))

        def emit_trans():
            p, ibx, it, osb = trans_due.pop(0)
            nc.sync.dma_start_transpose(
                oT[p][:, ibx * ib + it * 128: ibx * ib + (it + 1) * 128],
                osb)

        ysb_open = {}

        def emit_proj_group(it, c, final=False):
            if c == 0:
                ysb_open[it] = y_pool.tile([128, dim], BF16, name="ysb")
            ysb = ysb_open[it]
            if final:
                ps = ps_pool.tile([128, ib], F32, name="psS", tag="psS",
                                  bufs=2)
                ps = ps[:, 0:fc]
            else:
                ps = psC[:, 0:fc]
            for t in range(S):
                nc.tensor.matmul(
                    ps, oT[t][:, ts(it, 128)], wout_sb[t][:, ts(c, fc)],
                    start=(t == 0), stop=(t == S - 1),
                    skip_group_check=True)
            cp(ysb[:, ts(c, fc)], ps)
            if c == ncs - 1:
                nc.sync.dma_start(y[ts(it, 128), :], ysb)
                del ysb_open[it]

        # prefix: the idle po banks and psS slots join the transient
        # rotation until B claims them
        if itpb * dh >= 512:
            po_scr = [ps_pool.tile([128, itpb * dh], F32, name="po",
                                   tag=f"po{i}") for i in range(2)]
            ps_scr = [ps_pool.tile([128, ib], F32, name="psS", tag="psS",
                                   bufs=2) for _ in range(2)]
            set_ring([psC] + po_scr + ps_scr)
        pull_until(lambda: s_req(0))
        pull_until(lambda: s_req(1))
        cp_state["act_free"] = False
        if itpb * dh >= 512:
            # psS slots are claimed by the S stream now; po banks stay in
            # the ring until the first PV/den accumulation
            set_ring([psC] + po_scr)

        s_ahead = None
        n_steps = len(steps)
        for k in range(n_steps):
            ibx, h, jt = steps[k]
            a_budget["ns"] += 1600 if k < 16 else (1000 if k < 48 else 560)
            pull_until(lambda: s_req(k))
            if s_ahead is None:
                s_ahead = emit_s(k)
            psS = s_ahead
            pull_paced()
            # s_lead: S for step k+1 before exp(k) keeps ScalarE fed
            if k + 1 < n_steps and s_req(k + 1):
                s_ahead = emit_s(k + 1)
            else:
                s_ahead = None
            pexp = pexp_pool.tile([128, ib], BF16, name="pexp")
            nc.scalar.activation(pexp, psS, AF.Exp, scale=scale)
            pend.append((pexp, ibx, h, jt))
            pull_paced()
            # drain PV: keep the FIFO short when v is available; never let
            # it reach the pexp pool depth (emission-order deadlock).
            def v_ready():
                return pend and (pend[0][3] // 4 in adone["v"]
                                 or a_exhausted["v"])
            while pend and ((len(pend) > 6 and v_ready())
                            or len(pend) >= 12):
                if not v_ready():
                    pull_until(lambda: pend[0][3] // 4 in adone["v"])
                pop_pend()
            if s_ahead is None and k + 1 < n_steps:
                if s_req(k + 1):
                    s_ahead = emit_s(k + 1)
            pull_paced()
            # weave deferred transposes and projection slices
            if trans_due and k % 2 == 0:
                emit_trans()
            if proj_due and k % 4 == 1:
                emit_proj_group(*proj_due.pop(0))
            if jt == JT - 1 and h % 2 == 1 and h == hpc - 1:
                # end of an i-block: flush stragglers, queue projections
                while pend:
                    if not (pend[0][3] // 4 in adone["v"]):
                        pull_until(lambda: pend[0][3] // 4 in adone["v"])
                    pop_pend()
                while proj_due:
                    emit_proj_group(*proj_due.pop(0))
                proj_due = [(it, c)
                            for it in range(ibx * itpb, (ibx + 1) * itpb)
                            for c in range(ncs)]
                if ibx == n_ibx - 1:
                    while trans_due:
                        emit_trans()
                    cp_state["act_free"] = True   # exp stream is done
                    for it, c in proj_due:
                        emit_proj_group(it, c, final=True)
                    proj_due = []

        while pull_a():
            pass


_BUILD_CACHE = {}


def build_nc(n=N_FULL, dim=DIM_FULL, hpc=HPC, dh=DH, mm_dt=None, ib=1024,
             **unused):
    key = (n, dim, hpc, dh, ib)
    if key in _BUILD_CACHE:
        return _BUILD_CACHE[key]
    inner = hpc * dh
    nc = bacc.Bacc("TRN2", target_bir_lowering=False, debug=False)
    x = nc.dram_tensor("x", [n, dim], BF16, kind="ExternalInput").ap()
    wqkv = nc.dram_tensor("w_qkv", [dim, 3 * inner], BF16,
                          kind="ExternalInput").ap()
    wout = nc.dram_tensor("w_out", [inner, dim], BF16,
                          kind="ExternalInput").ap()
    y = nc.dram_tensor("y", [n, dim], BF16, kind="ExternalOutput").ap()
    with tile.TileContext(nc) as tc:
        with nc.allow_low_precision(
                reason="bf16 feeds; fp32 PSUM accumulation"):
            emit_core_kernel(nc, tc, x, wqkv, wout, y, n=n, dim=dim,
                             hpc=hpc, dh=dh, ib=ib)
    nc.compile()
    _BUILD_CACHE[key] = nc
    return nc


def make_in_maps(x, w_qkv, w_out):
    """Shard full inputs into the 8 per-core input maps."""
    x = np.asarray(x, dtype=np.float32)
    w_qkv = np.asarray(w_qkv, dtype=np.float32)
    w_out = np.asarray(w_out, dtype=np.float32)
    qk_off = HEADS_FULL * DH          # 1024: start of K block in w_qkv
    in_maps = []
    for c in range(N_CORES):
        b, g = divmod(c, GROUPS)
        cols = ts(g, INNER_PC)
        wq = w_qkv[:, cols]
        wk = w_qkv[:, qk_off + g * INNER_PC: qk_off + (g + 1) * INNER_PC]
        wv = w_qkv[:, 2 * qk_off + g * INNER_PC: 2 * qk_off + (g + 1) * INNER_PC]
        in_maps.append({
            "x": np.ascontiguousarray(x[b]).astype(ml_dtypes.bfloat16),
            "w_qkv": np.ascontiguousarray(
                np.concatenate([wq, wk, wv], axis=1)).astype(
                    ml_dtypes.bfloat16),
            "w_out": np.ascontiguousarray(w_out[cols, :]).astype(
                ml_dtypes.bfloat16),
        })
    return in_maps


def kernel(x, w_qkv, w_out, b_out, trace=False):
    b_out = np.asarray(b_out, dtype=np.float32)
    nc = build_nc()
    in_maps = make_in_maps(x, w_qkv, w_out)
    res = bass_utils.run_bass_kernel_spmd(
        nc, in_maps, core_ids=list(range(N_CORES)), trace=trace)
    ys = [np.asarray(r["y"], dtype=np.float32) for r in res.results]
    out = np.empty((B_FULL, N_FULL, DIM_FULL), dtype=np.float32)
    for b in range(B_FULL):
        out[b] = ys[GROUPS * b] + ys[GROUPS * b + 1] + b_out[None, :]
    if trace:
        kernel.last_result = res
    return out


# revision 101
# speedup vs baseline: 1.1485x; 1.0335x over previous
"""Multi-head attention (b=4, n=2048, dim=1024, 16 heads x 64) on 8 Trainium2
NeuronCores.

Sharding: data-parallel over batch (4) x tensor-parallel over head-groups (2).
Each core gets one batch element and 8 heads; the host sums the two
head-group partials per batch element and adds b_out.

Per-core pipeline (bf16 dataflow, fp32 PSUM accumulation):
  A (lazily woven into B): x^T lands in SBUF via XBAR DMA transposes
      (14ns per 16x128 tile -- no PE time, no PSUM, no drain copies);
      the qkv strips are full-bank PSUM groups that rotate through the
      idle PSUM banks during the prefix and through the psC bank
      (spread across steps) once B owns the others.  The first four
      strips accumulate kc-major, each matmul firing as its w strip's
      DMA lands, so the first exp runs ~25us in.  Dummy transposes
      keep the PE at full p-state through the DMA-bound prefix.
  B:  steps ordered (i-block, head-PAIR, jt, head) -- the pair
      interleave doubles the time the A-weave has to deliver each kT/v
      chunk: S^T j-tiles = matmul(lhsT=kT_h j-block, rhs=qT_h i-block)
      -> psS [128, ib] (tag bufs=2); exp on ScalarE (1/sqrt(dh) folded
      into the activation scale) -> pexp bf16; PV in the output-natural
      layout po[i, dh] (lhsT=pexp i-slices, rhs=v bf16, N=dh -> full
      128-partition output fill, HALF the PE time of the [dh, i]
      layout); denominators via N=1 matmuls into a dedicated bank.
      PSUM zero-region semantics let many accumulation groups share a
      bank: only the first matmul of a bank's first group uses
      start=True; later groups ride the pending-zero region
      (skip_group_check).  PV trails exp through a FIFO that grows
      while A hasn't delivered the needed v chunk, so ScalarE (the
      kernel bottleneck at ~267us busy) never stalls on phase A.
      Tail per head: DVE reciprocal of the denominator column and a
      fused normalize+copy (tensor_scalar mult by a per-partition
      scalar) -> O_sb bf16; per pair, O_sb returns to oT strips via
      XBAR DMA transposes.
  C:  y = O @ w_out via lhsT = oT strips, woven through the NEXT
      i-block's steps on the psC bank; the final i-block's flush
      pipelines through the freed psS slots, with PSUM->SBUF copies
      alternating DVE/ScalarE (idle once the exp stream ends).
  x, w_qkv, w_out, y are all bf16 over the wire (the DMA pipe is a
  single serial ~360 B/ns resource; halving the bytes halves the
  startup and drain), accumulation in fp32 PSUM throughout.
"""

from contextlib import ExitStack

import numpy as np
import ml_dtypes

import concourse.mybir as mybir
import concourse.tile as tile
from concourse import bacc, bass_utils

F32 = mybir.dt.float32
BF16 = mybir.dt.bfloat16
AF = mybir.ActivationFunctionType
MUL = mybir.AluOpType.mult

# Full-problem constants (hardcoded per the harness contract).
B_FULL, N_FULL, DIM_FULL = 4, 2048, 1024
HEADS_FULL, DH = 16, 64
N_CORES = 8
GROUPS = 2                       # head-group (tensor-parallel) factor
HPC = HEADS_FULL // GROUPS       # heads per core = 8
INNER_PC = HPC * DH              # per-core inner dim = 512


def ts(i, size):
    return slice(i * size, (i + 1) * size)


def emit_core_kernel(nc, tc, x, wqkv, wout, y, *, n, dim, hpc, dh, ib=1024):
    inner = hpc * dh
    KC = dim // 128          # contraction chunks for the qkv projection
    S = inner // 128         # 128-row strips of the per-core inner dim
    JT = n // 128            # key/value j-tiles
    NB = n // 512            # 512-wide n-chunks in phase A
    PAIRS = hpc // 2
    ib = min(ib, n)
    n_ibx = n // ib
    itpb = ib // 128                 # i-tiles per i-block
    cpb = ib // 512                  # 512-chunks per i-block
    assert n % 512 == 0 and dim % 128 == 0 and inner % 128 == 0
    assert ib % 512 == 0 and n % ib == 0 and hpc % 2 == 0
    scale = float(1.0 / np.sqrt(dh))
    fc = min(512, dim)
    ncs = dim // fc                  # projection col slices per i-tile

    stack = ExitStack()
    with stack:
        const_pool = stack.enter_context(tc.tile_pool(name="const", bufs=1))
        persist = stack.enter_context(tc.tile_pool(name="persist", bufs=1))

        # ---- constants ----
        ident_f = const_pool.tile([128, 128], F32, name="ident_f")
        ident_src = nc.inline_tensor(np.eye(128, dtype=np.float32),
                                     name=f'identc{nc.next_id()}').ap()
        nc.scalar.dma_start(ident_f, ident_src)
        ident = const_pool.tile([128, 128], BF16, name="ident")
        nc.vector.tensor_copy(ident, ident_f)
        ident_r = ident_f.bitcast(mybir.dt.float32r)
        ones_col = const_pool.tile([128, 1], BF16, name="ones_col")
        nc.gpsimd.memset(ones_col, 1.0)

        # ---- persistent SBUF tensors ----
        kT = [persist.tile([128, n], BF16, name="kTs", tag=f"kT{s}")
              for s in range(S)]
        qT = [persist.tile([128, n], BF16, name="qTs", tag=f"qT{s}")
              for s in range(S)]
        oT = [persist.tile([128, n], BF16, name="oTs", tag=f"oT{s}")
              for s in range(S)]
        v_sb = [persist.tile([128, inner], BF16, name="vts", tag=f"v{jt}")
                for jt in range(JT)]
        # xtsF[kc]: transposed x (all n), contraction rows kc*128
        xtsF = [persist.tile([128, n], BF16, name="xts", tag=f"xts{kc}")
                for kc in range(KC)]
        w_sb = [persist.tile([128, 3 * inner], BF16, name="wt",
                             tag=f"w{kc}") for kc in range(KC)]
        wout_sb = [persist.tile([128, dim], BF16, name="wo", tag=f"wo{t}")
                   for t in range(S)]

        pexp_pool = stack.enter_context(tc.tile_pool(name="b_pexp",
                                                     bufs=14))
        osb_pool = stack.enter_context(tc.tile_pool(name="b_osb", bufs=2))
        tail_pool = stack.enter_context(tc.tile_pool(name="b_tail", bufs=4))
        y_pool = stack.enter_context(tc.tile_pool(name="c_y", bufs=3))

        ps_pool = stack.enter_context(
            tc.tile_pool(name="ps", bufs=1, space="PSUM"))
        # psS: 2 x [128, ib] (4 banks with ib=1024); po: 2 x [128, 512]
        # (1 bank each); den: [128, 128] (1 bank); psC: [128, 512] (1 bank).
        den_ps = ps_pool.tile([128, 128], F32, name="den_ps", tag="den")
        psC = ps_pool.tile([128, 512], F32, name="psC", tag="psC")

        # weight DMAs are emitted inside a_items (interleaved with x) so
        # the SP sequencer's ~650ns-per-DMA issue cost doesn't serialize
        # the startup critical path.

        # ---------- transient-PSUM bank ring ----------
        # The tile framework tracks PSUM deps at whole-tile granularity, so
        # consecutive transient groups in ONE bank serialize against each
        # other's drain copies.  During the prefix (before any PV/den
        # accumulation), the idle po banks join psC in a rotation, which
        # fully pipelines the x-transposes and first qkv strips.  Once B
        # starts, transients use only psC and are spread across the
        # Act-bound steps so the drain latency hides.
        ring = {"banks": [psC], "i": 0, "r": 0}

        def set_ring(banks):
            ring["banks"] = banks
            ring["i"] = 0
            ring["r"] = 0
            ring["cur"] = banks[0]

        def next_bank():
            b = ring["banks"][ring["i"] % len(ring["banks"])]
            ring["i"] += 1
            return b

        def next_tregion(dt=BF16):
            # one [128, 128] transpose slot; a fresh BANK every call —
            # whole-tile dep tracking makes same-bank reuse serialize
            # against the previous op's drain copy
            w = 64 if dt == BF16 else 128
            return next_bank()[:, 0:w].bitcast(dt)

        # ---------- phase A as a lazily-pulled item stream ----------
        adone = {"qT": set(), "kT": set(), "v": set(), "xts": set()}
        cp_state = {"act_free": True, "flip": 0}

        def cp(dst, src, allow_act=True):
            # before the exp stream starts (and after it ends) the Act
            # engine is idle; route alternate drain copies there
            if cp_state["act_free"] and allow_act:
                cp_state["flip"] ^= 1
                if cp_state["flip"]:
                    nc.scalar.copy(dst, src)
                    return
            nc.vector.tensor_copy(dst, src)

        def a_items():
            dq_state = {"i": 0}

            def dq():
                # all phase-A loads ride the SP queue back-to-back: same-
                # queue DMAs pipeline at the ~650ns issue rate, while
                # cross-queue interleaving serializes on completion order
                return nc.sync

            def fetch_xts(c0, c1, kcs=None):
                # XBAR DMA transpose: x[c0..c1 chunks, kc cols] ->
                # xtsF[kc][:, span] — one DMA per kc spanning several
                # chunks (the shared HWDGE costs ~625ns per DMA, so fewer
                # bigger transfers win)
                c1 = min(c1, NB)
                if c0 >= c1:
                    return
                for kc in (range(KC) if kcs is None else kcs):
                    dq().dma_start_transpose(
                        xtsF[kc][:, c0 * 512: c1 * 512],
                        x[c0 * 512: c1 * 512, ts(kc, 128)])
                for c in range(c0, c1):
                    adone["xts"].add(c)

            # PE warmup: dummy transposes keep the array busy from t~2us
            # so the first strips run at full p-state (idle drops the PE
            # to 2-4x slower p-states); results are never read.
            for i in range(220):
                nc.tensor.matmul(psC[:, 0:64].bitcast(BF16), ident, ident,
                                 is_transpose=True, skip_group_check=True)

            # startup DMA order: all first-half x^T transposes, then all
            # q/k weight strips — same-type DMAs pipeline at the ~650ns
            # issue rate while mixed types serialize on completion
            fetch_xts(0, min(2, NB))
            for kc in range(KC):
                dq().dma_start(w_sb[kc][:, 0:2 * inner],
                               wqkv[ts(kc, 128), 0:2 * inner])

            def transposes(c):
                if c < NB and c not in adone["xts"]:
                    fetch_xts(c, c + 2)
                    yield 100

            def qk_strip(which, s, c):
                # qT/kT strip s over one 512-chunk: one full-bank group
                base = which * inner + s * 128
                dst = (qT if which == 0 else kT)[s]
                ring["r"] = 0
                slot = next_bank()
                for kc in range(KC):
                    nc.tensor.matmul(
                        slot[:, 0:512], w_sb[kc][:, base:base + 128],
                        xtsF[kc][:, ts(c, 512)],
                        start=(kc == 0), stop=(kc == KC - 1),
                        skip_group_check=True)
                cp(dst[:, ts(c, 512)], slot[:, 0:512],
                   allow_act=(s == 0 and c < cpb))
                yield KC * 215
                adone["qT" if which == 0 else "kT"].add((s, c))

            def v_tile(c, j2):
                it = c * 4 + j2
                ring["r"] = 0
                slot = next_bank()
                for kc in range(KC):
                    nc.tensor.matmul(
                        slot[:, 0:inner],
                        xtsF[kc][:, c * 512 + j2 * 128:
                                  c * 512 + (j2 + 1) * 128],
                        w_sb[kc][:, 2 * inner: 3 * inner],
                        start=(kc == 0), stop=(kc == KC - 1),
                        skip_group_check=True)
                nc.vector.tensor_copy(v_sb[it], slot[:, 0:inner])
                yield KC * inner * 0.42
                if j2 == 3:
                    adone["v"].add(c)

            # demand-ordered: pair 0's q/k first (kT needs every chunk
            # within the first pair-block), then v chunks interleaved
            # with later pairs' strips; later-i-block q chunks last.
            # Chunk c+1's transposes interleave with chunk c's strips so
            # the PE strip work overlaps the DVE/Act transpose drains.
            tq = {c: transposes(c) for c in range(NB)}

            def t_weave(c, k2=2):
                if c < NB:
                    for _ in range(k2):
                        if next(tq[c], "E") != "E":
                            yield 430

            if NB >= 2 and cpb == 2 and len(ring["banks"]) >= 5:
                # fast prefix: the four s0 strip groups accumulate
                # kc-major on dedicated banks, paced by the w arrivals
                sbank = ring["banks"][1:5]
                descs = [(0, 0), (1, 0), (0, 1), (1, 1)]  # (which, c)
                for kc in range(KC):
                    for g, (which, c) in enumerate(descs):
                        base = which * inner
                        nc.tensor.matmul(
                            sbank[g][:, 0:512],
                            w_sb[kc][:, base:base + 128],
                            xtsF[kc][:, ts(c, 512)],
                            start=(kc == 0), stop=(kc == KC - 1),
                            skip_group_check=True)
                    yield 860
                tq[0] = tq[1] = iter(())
                for g, (which, c) in enumerate(descs):
                    dst = (qT if which == 0 else kT)[0]
                    cp(dst[:, ts(c, 512)], sbank[g][:, 0:512])
                    adone["qT" if which == 0 else "kT"].add((0, c))
                yield 200
                fetch_xts(2, NB)
                for t in range(S):
                    nc.sync.dma_start(wout_sb[t], wout[ts(t, 128), :])
                for kc in range(KC):
                    nc.sync.dma_start(
                        w_sb[kc][:, 2 * inner: 3 * inner],
                        wqkv[ts(kc, 128), 2 * inner: 3 * inner])
                for c in range(2, NB):
                    yield from qk_strip(1, 0, c)
                for c in range(min(2, NB)):
                    for j2 in range(4):
                        yield from v_tile(c, j2)
            else:
                yield from tq[0]
                for c in range(NB):
                    if c < cpb:
                        yield from t_weave(c + 1)
                        yield from qk_strip(0, 0, c)
                    yield from t_weave(c + 1)
                    yield from qk_strip(1, 0, c)
                    yield from t_weave(c + 1, 4)
                    if c == min(1, NB - 1):
                        for t in range(S):
                            nc.sync.dma_start(wout_sb[t],
                                              wout[ts(t, 128), :])
            strip_q = [("q", s, c) for s in range(1, S) for c in range(cpb)]
            strip_k = [("k", s, c) for s in range(1, S) for c in range(NB)]
            later = []
            for s in range(1, S):
                later.extend([("q", s, c) for c in range(cpb)])
                later.extend([("k", s, c) for c in range(NB)])
            if not any(2 * inner <= 3 * inner and c in adone["v"]
                       for c in range(NB)):
                for kc in range(KC):
                    nc.sync.dma_start(
                        w_sb[kc][:, 2 * inner: 3 * inner],
                        wqkv[ts(kc, 128), 2 * inner: 3 * inner])
            for c in range(NB):
                if c in adone["v"]:
                    continue
                for j2 in range(4):
                    yield from v_tile(c, j2)
                take, later = later[:cpb + 1], later[cpb + 1:]
                for kind, s, c2 in take:
                    yield from qk_strip(0 if kind == "q" else 1, s, c2)
            for kind, s, c2 in later:
                yield from qk_strip(0 if kind == "q" else 1, s, c2)
            for s in range(S):
                for c in range(cpb, min(NB, cpb * n_ibx)):
                    if (s, c) not in adone["qT"]:
                        yield from qk_strip(0, s, c)

        agen = a_items()
        a_exhausted = {"v": False}
        a_budget = {"ns": 0.0}

        def pull_a():
            """Pull one A item; returns False when exhausted."""
            cost = next(agen, None)
            if cost is None:
                a_exhausted["v"] = True
                return False
            a_budget["ns"] -= cost
            return True

        def pull_paced():
            if a_budget["ns"] > 0 and not a_exhausted["v"]:
                pull_a()

        def pull_until(pred):
            while not pred():
                if not pull_a():
                    assert pred(), "phase A exhausted without satisfying dep"
                    return

        # ---------- phase B/C emission ----------
        # steps ordered (ibx, pair, jt, h-in-pair)
        steps = [(ibx, 2 * p + hi, jt)
                 for ibx in range(n_ibx) for p in range(PAIRS)
                 for jt in range(JT) for hi in range(2)]

        def s_req(k):
            ibx, h, jt = steps[k]
            s = h // 2
            return (all((s, c) in adone["qT"]
                        for c in range(ibx * cpb, (ibx + 1) * cpb))
                    and (s, jt // 4) in adone["kT"])

        pend = []                 # FIFO of (pexp, ibx, h, jt)
        po_tiles = {}             # h -> po tile for the live pair
        trans_due = []            # deferred (pair, ibx, it, osb) transposes
        proj_due = []
        osb_tiles = {}

        def emit_s(k):
            ibx, h, jt = steps[k]
            s, r = divmod(h * dh, 128)
            psS = ps_pool.tile([128, ib], F32, name="psS", tag="psS",
                               bufs=2)
            for c in range(cpb):
                nc.tensor.matmul(
                    psS[:, ts(c, 512)], kT[s][r:r + dh, ts(jt, 128)],
                    qT[s][r:r + dh, ibx * ib + c * 512: ibx * ib + (c + 1) * 512],
                    start=True, stop=True)
            return psS

        def pop_pend():
            if ring["banks"][0] is not psC or len(ring["banks"]) > 1:
                set_ring([psC])      # PV/den claim the po banks now
            pexp, ibx, h, jt = pend.pop(0)
            if jt == 0:
                po_tiles[h] = ps_pool.tile(
                    [128, itpb * dh], F32, name="po", tag=f"po{h % 2}")
            po = po_tiles[h]
            first_of_pair = (h % 2 == 0 and jt == 0)
            for it in range(itpb):
                lhsT = pexp[:, ts(it, 128)]
                nc.tensor.matmul(
                    po[:, ts(it, dh)], lhsT, v_sb[jt][:, ts(h, dh)],
                    start=(jt == 0 and it == 0), stop=(jt == JT - 1),
                    skip_group_check=True)
                nc.tensor.matmul(
                    den_ps[:, (ibx % 2) * 64 + (h % 2) * itpb + it:
                           (ibx % 2) * 64 + (h % 2) * itpb + it + 1],
                    lhsT, ones_col,
                    start=(first_of_pair and it == 0), stop=(jt == JT - 1),
                    skip_group_check=True)
            if jt == JT - 1:
                emit_tail(ibx, h)

        def emit_tail(ibx, h):
            po = po_tiles.pop(h)
            p = h // 2
            for it in range(itpb):
                dcol = (ibx % 2) * 64 + (h % 2) * itpb + it
                recip = tail_pool.tile([128, 1], F32, name="recip")
                nc.vector.reciprocal(recip, den_ps[:, dcol:dcol + 1])
                if h % 2 == 0:
                    osb_tiles[it] = osb_pool.tile([128, 128], BF16,
                                                  name="osb", tag=f"osb{it}")
                nc.vector.tensor_scalar(
                    osb_tiles[it][:, (h % 2) * dh: (h % 2 + 1) * dh],
                    po[:, ts(it, dh)], recip, None, MUL)
            if h % 2 == 1:
                for it in range(itpb):
                    trans_due.append((p, ibx, it, osb_tiles.pop(it)))

        def emit_trans():
            p, ibx, it, osb = trans_due.pop(0)
            nc.sync.dma_start_transpose(
                oT[p][:, ibx * ib + it * 128: ibx * ib + (it + 1) * 128],
                osb)

        ysb_open = {}

        def emit_proj_group(it, c, final=False):
            if c == 0:
                ysb_open[it] = y_pool.tile([128, dim], BF16, name="ysb")
            ysb = ysb_open[it]
            if final:
                ps = ps_pool.tile([128, ib], F32, name="psS", tag="psS",
                                  bufs=2)
                ps = ps[:, 0:fc]
            else:
                ps = psC[:, 0:fc]
            for t in range(S):
                nc.tensor.matmul(
                    ps, oT[t][:, ts(it, 128)], wout_sb[t][:, ts(c, fc)],
                    start=(t == 0), stop=(t == S - 1),
                    skip_group_check=True)
            cp(ysb[:, ts(c, fc)], ps)
            if c == ncs - 1:
                nc.sync.dma_start(y[ts(it, 128), :], ysb)
                del ysb_open[it]

        # prefix: the idle po banks and psS slots join the transient
        # rotation until B claims them
        if itpb * dh >= 512:
            po_scr = [ps_pool.tile([128, itpb * dh], F32, name="po",
                                   tag=f"po{i}") for i in range(2)]
            ps_scr = [ps_pool.tile([128, ib], F32, name="psS", tag="psS",
                                   bufs=2) for _ in range(2)]
            set_ring([psC] + po_scr + ps_scr)
        pull_until(lambda: s_req(0))
        pull_until(lambda: s_req(1))
        cp_state["act_free"] = False
        if itpb * dh >= 512:
            # psS slots are claimed by the S stream now; po banks stay in
            # the ring until the first PV/den accumulation
            set_ring([psC] + po_scr)

        s_ahead = None
        n_steps = len(steps)
        for k in range(n_steps):
            ibx, h, jt = steps[k]
            a_budget["ns"] += 620
            pull_until(lambda: s_req(k))
            if s_ahead is None:
                s_ahead = emit_s(k)
            psS = s_ahead
            pull_paced()
            # s_lead: S for step k+1 before exp(k) keeps ScalarE fed
            if k + 1 < n_steps and s_req(k + 1):
                s_ahead = emit_s(k + 1)
            else:
                s_ahead = None
            pexp = pexp_pool.tile([128, ib], BF16, name="pexp")
            nc.scalar.activation(pexp, psS, AF.Exp, scale=scale)
            pend.append((pexp, ibx, h, jt))
            pull_paced()
            # drain PV: keep the FIFO short when v is available; never let
            # it reach the pexp pool depth (emission-order deadlock).
            def v_ready():
                return pend and (pend[0][3] // 4 in adone["v"]
                                 or a_exhausted["v"])
            while pend and ((len(pend) > 6 and v_ready())
                            or len(pend) >= 12):
                if not v_ready():
                    pull_until(lambda: pend[0][3] // 4 in adone["v"])
                pop_pend()
            if s_ahead is None and k + 1 < n_steps:
                if s_req(k + 1):
                    s_ahead = emit_s(k + 1)
            pull_paced()
            # weave deferred transposes and projection slices
            if trans_due and k % 2 == 0:
                emit_trans()
            if proj_due and k % 4 == 1:
                emit_proj_group(*proj_due.pop(0))
            if jt == JT - 1 and h % 2 == 1 and h == hpc - 1:
                # end of an i-block: flush stragglers, queue projections
                while pend:
                    if not (pend[0][3] // 4 in adone["v"]):
                        pull_until(lambda: pend[0][3] // 4 in adone["v"])
                    pop_pend()
                while proj_due:
                    emit_proj_group(*proj_due.pop(0))
                proj_due = [(it, c)
                            for it in range(ibx * itpb, (ibx + 1) * itpb)
                            for c in range(ncs)]
                if ibx == n_ibx - 1:
                    while trans_due:
                        emit_trans()
                    cp_state["act_free"] = True   # exp stream is done
                    for it, c in proj_due:
                        emit_proj_group(it, c, final=True)
                    proj_due = []

        while pull_a():
            pass


_BUILD_CACHE = {}


def build_nc(n=N_FULL, dim=DIM_FULL, hpc=HPC, dh=DH, mm_dt=None, ib=1024,
             **unused):
    key = (n, dim, hpc, dh, ib)
    if key in _BUILD_CACHE:
        return _BUILD_CACHE[key]
    inner = hpc * dh
    nc = bacc.Bacc("TRN2", target_bir_lowering=False, debug=False)
    x = nc.dram_tensor("x", [n, dim], BF16, kind="ExternalInput").ap()
    wqkv = nc.dram_tensor("w_qkv", [dim, 3 * inner], BF16,
                          kind="ExternalInput").ap()
    wout = nc.dram_tensor("w_out", [inner, dim], BF16,
                          kind="ExternalInput").ap()
    y = nc.dram_tensor("y", [n, dim], BF16, kind="ExternalOutput").ap()
    with tile.TileContext(nc) as tc:
        with nc.allow_low_precision(
                reason="bf16 feeds; fp32 PSUM accumulation"):
            emit_core_kernel(nc, tc, x, wqkv, wout, y, n=n, dim=dim,
                             hpc=hpc, dh=dh, ib=ib)
    nc.compile()
    _BUILD_CACHE[key] = nc
    return nc


def make_in_maps(x, w_qkv, w_out):
    """Shard full inputs into the 8 per-core input maps."""
    x = np.asarray(x, dtype=np.float32)
    w_qkv = np.asarray(w_qkv, dtype=np.float32)
    w_out = np.asarray(w_out, dtype=np.float32)
    qk_off = HEADS_FULL * DH          # 1024: start of K block in w_qkv
    in_maps = []
    for c in range(N_CORES):
        b, g = divmod(c, GROUPS)
        cols = ts(g, INNER_PC)
        wq = w_qkv[:, cols]
        wk = w_qkv[:, qk_off + g * INNER_PC: qk_off + (g + 1) * INNER_PC]
        wv = w_qkv[:, 2 * qk_off + g * INNER_PC: 2 * qk_off + (g + 1) * INNER_PC]
        in_maps.append({
            "x": np.ascontiguousarray(x[b]).astype(ml_dtypes.bfloat16),
            "w_qkv": np.ascontiguousarray(
                np.concatenate([wq, wk, wv], axis=1)).astype(
                    ml_dtypes.bfloat16),
            "w_out": np.ascontiguousarray(w_out[cols, :]).astype(
                ml_dtypes.bfloat16),
        })
    return in_maps


def kernel(x, w_qkv, w_out, b_out, trace=False):
    b_out = np.asarray(b_out, dtype=np.float32)
    nc = build_nc()
    in_maps = make_in_maps(x, w_qkv, w_out)
    res = bass_utils.run_bass_kernel_spmd(
        nc, in_maps, core_ids=list(range(N_CORES)), trace=trace)
    ys = [np.asarray(r["y"], dtype=np.float32) for r in res.results]
    out = np.empty((B_FULL, N_FULL, DIM_FULL), dtype=np.float32)
    for b in range(B_FULL):
        out[b] = ys[GROUPS * b] + ys[GROUPS * b + 1] + b_out[None, :]
    if trace:
        kernel.last_result = res
    return out
